# revision 39
# baseline (speedup 1.0000x reference)
"""Deformable-align kernel for 8 TRN2 NeuronCores.

Strategy: data-parallel, core i handles batch b=i//2, row-half r=i%2
(32 rows x 64 cols = 2048 pixels). No collectives.

v2 pipeline (s-block-pipelined, PE-weighted bilinear):
  - Host builds a ZERO-PADDED corner table xq[4489, 1024] over the 67x67
    grid (gy,gx in [-2,64]): row = [a, b, c, d] x (chi,c_lo) where
    a=z00, b=z01-z00, c=z10-z00, d=z11-z01-z10+z00 and out-of-bounds
    corners are 0. Zero padding makes the bilinear weights exact pure
    fractions (sum-to-1), so per (tap,block):
        P = a + wx*b + wy*(c + wx*d)
  - head runs per s-block of 128 px: offset-conv (18 shifted matmuls)
    -> +bias -> PE-transpose of the [18, 128] offset chunk -> floor/
    clip/weights -> int32 row index; the first gathers start ~7us in.
  - gathers: one indirect SWDGE DMA per (tap, s-block) with [128, 1]
    per-partition row offsets, 2KB rows (multi-offset indirect DMA is
    broken in the real SWDGE ucode - verified; this is the fastest
    correct form). Pool engine issue cost (144 x ~0.8us) and the DMA
    device (37.7MB of gather traffic) are the co-bottlenecks.
  - weighting+transpose fused on PE: per (tap, s-block, chi) 3 matmuls
    [ident*a + diag(wx)*b + diag(wy)*u] accumulate into a psum slab
    [c, px]; u = c + wx*d is one DVE scalar_tensor_tensor; diag tiles
    are cheap DVE tensor_scalar on a f16 identity.
  - main matmul runs in closed partial groups (after taps 2/5/7/8) so
    the last tap leaves only a 4-matmul tail; partials accumulate in
    SBUF via DVE, bias folded into the first partial, out stored f16
    (host casts to f32; ~3e-5 extra rel err).
  - weight load for the main matmul is delayed behind a dummy WAW dep
    to keep the early DMA-device window free for the first gathers.
"""
import sys

import numpy as np

sys.path.insert(0, "/opt/trn_rl_repo")

import concourse.bass as bass
import concourse.tile as tile
from concourse import bacc, mybir
from concourse.bass_utils import run_bass_kernel_spmd

F16 = mybir.dt.float16
F32 = mybir.dt.float32
I32 = mybir.dt.int32
AF = mybir.ActivationFunctionType
OP = mybir.AluOpType

B, C, O, H, W = 4, 256, 256, 64, 64
K2 = 9
HALF = 32
PX = HALF * W            # 2048 per core
GRID = 67                # padded grid side (gy,gx in [-2,64])
NROW = GRID * GRID       # 4489
MAGIC = 12582912.0       # 1.5 * 2**23

_cache = {}


def build_nc():
    nc = bacc.Bacc(
        "TRN2", target_bir_lowering=False, debug=False,
        enable_asserts=False, num_devices=8,
    )
    xq = nc.dram_tensor("xq", [NROW, 1024], F16, kind="ExternalInput")
    ypad = nc.dram_tensor("ypad", [128, 4488], F16, kind="ExternalInput")
    offw = nc.dram_tensor("offw", [128, 324], F16, kind="ExternalInput")
    dwt = nc.dram_tensor("dwt", [128, 4608], F16, kind="ExternalInput")
    offb = nc.dram_tensor("offb", [18, 1], F32, kind="ExternalInput")
    dbias = nc.dram_tensor("dbias", [128, 2], F32, kind="ExternalInput")
    basey = nc.dram_tensor("basey", [128, 144], F32, kind="ExternalInput")
    basex = nc.dram_tensor("basex", [128, 144], F32, kind="ExternalInput")
    ident = nc.dram_tensor("ident", [128, 128], F16, kind="ExternalInput")
    out = nc.dram_tensor("out", [256, 2048], F16, kind="ExternalOutput")

    with tile.TileContext(nc) as tc:
        build_body(tc, xq, ypad, offw, dwt, offb, dbias, basey, basex,
                   ident, out)
    nc.compile()
    return nc


def _sub(ap, off, dims):
    """Custom sub-AP of a tile AP: extra element offset + explicit free dims."""
    part = list(ap.ap[0])
    return bass.AP(ap.tensor, ap.offset + off, [part] + [list(d) for d in dims])


def build_body(tc, xq, ypad, offw, dwt, offb, dbias, basey, basex, ident,
               out):
    nc = tc.nc
    from contextlib import ExitStack

    xq_ap = bass.AP(xq, 0, [[1024, NROW], [1, 1024]])

    with ExitStack() as ctx:
        cpool = ctx.enter_context(tc.tile_pool(name="consts", bufs=1))
        yp = cpool.tile([128, 4488], F16)
        nc.sync.dma_start(_sub(yp[:], 0, [[2244, 2], [1, 462]]),
                          _sub(ypad.ap(), 0, [[2244, 2], [1, 462]]))
        ow = cpool.tile([128, 324], F16)
        nc.sync.dma_start(ow[:], offw.ap())
        ob = cpool.tile([18, 1], F32)
        nc.sync.dma_start(ob[:], offb.ap())
        by = cpool.tile([128, 144], F32)
        nc.sync.dma_start(by[:], basey.ap())
        bx = cpool.tile([128, 144], F32)
        nc.sync.dma_start(bx[:], basex.ap())
        idt = cpool.tile([128, 128], F16)
        nc.sync.dma_start(idt[:], ident.ap())
        nc.sync.dma_start(_sub(yp[:], 462, [[2244, 2], [1, 1782]]),
                          _sub(ypad.ap(), 462, [[2244, 2], [1, 1782]]))
        idf = cpool.tile([18, 18], F32)
        nc.vector.tensor_copy(idf[:], idt[0:18, 0:18])
        db = cpool.tile([128, 2], F32)
        nc.sync.dma_start(db[:], dbias.ap())
        dw = cpool.tile([128, 4608], F16)

        ppool = ctx.enter_context(tc.tile_pool(name="persist", bufs=1))
        wyt = ppool.tile([128, 144], F32)
        wxt = ppool.tile([128, 144], F32)
        icol = ppool.tile([128, 144], I32)

        cps = ctx.enter_context(tc.tile_pool(name="cps", bufs=1, space="PSUM"))
        tps = ctx.enter_context(tc.tile_pool(name="tps", bufs=1, space="PSUM"))
        s2 = ctx.enter_context(tc.tile_pool(name="s2", bufs=2))
        tmp = ctx.enter_context(tc.tile_pool(name="tmp", bufs=2))
        gpool = ctx.enter_context(tc.tile_pool(name="gpool", bufs=6))
        wpool = ctx.enter_context(tc.tile_pool(name="wpool", bufs=8))
        slabs = ctx.enter_context(tc.tile_pool(name="slabs", bufs=2,
                                               space="PSUM"))
        qpool = ctx.enter_context(tc.tile_pool(name="qpool", bufs=12))
        outps = ctx.enter_context(tc.tile_pool(name="outps", bufs=1,
                                               space="PSUM"))
        bpool = ctx.enter_context(tc.tile_pool(name="bpool", bufs=2))

        def head_quarter(qc):
            # whole chain runs per s-block so the first gathers only wait on
            # s-block 0 of quarter 0 (conv -> bias -> PE transpose -> idx)
            pq = cps.tile([18, 512], F32, name="pq", tag="pq")
            pT = tps.tile([128, 72], F32, name="pT", tag="pT")
            for sl in range(4):
                s = 4 * qc + sl
                n = 0
                for ki in range(3):
                    for kj in range(3):
                        for chi in range(2):
                            rhs = _sub(yp, chi * 2244 + (ki + s * 2) * 66 + kj,
                                       [[66, 2], [1, 64]])
                            lhsT = _sub(ow, ((ki * 3 + kj) * 2 + chi) * 18,
                                        [[1, 18]])
                            nc.tensor.matmul(
                                pq[:, sl * 128:(sl + 1) * 128], lhsT, rhs,
                                start=(n == 0), stop=(n == 17))
                            n += 1
                offs = s2.tile([18, 128], F32, name="offs", tag="offs")
                nc.vector.tensor_scalar(offs[:],
                                        pq[:, sl * 128:(sl + 1) * 128],
                                        ob[:, 0:1], None, OP.add)
                nc.tensor.transpose(pT[:, sl * 18:(sl + 1) * 18],
                                    offs[:], idf[:])

                # indices + weights for this s-block; wyt/wxt keep
                # [p, k*16+s] cols, icol cols = qc*36 + k*4 + sl
                kd = [[16, K2]]
                offc = 4 * qc + sl

                def axis(dcol, base, wt_, kind):
                    v = tmp.tile([128, K2], F32, name=f"v{kind}",
                                 tag=f"v{kind}")
                    nc.vector.tensor_add(v[:],
                                         _sub(pT, sl * 18 + dcol, [[2, K2]]),
                                         _sub(base, offc, kd))
                    t = tmp.tile([128, K2], F32, name=f"t{kind}",
                                 tag=f"t{kind}")
                    nc.vector.tensor_scalar(t[:], v[:], -0.5, MAGIC, OP.add,
                                            OP.add)
                    g = tmp.tile([128, K2], F32, name=f"g{kind}",
                                 tag=f"g{kind}")
                    nc.vector.tensor_scalar(g[:], t[:], -MAGIC, 66.0, OP.add,
                                            OP.min)
                    nc.vector.tensor_scalar(g[:], g[:], 0.0, None, OP.max)
                    nc.vector.tensor_sub(_sub(wt_, offc, kd), v[:], g[:])
                    return g

                gy = axis(0, by, wyt, "y")
                gx = axis(1, bx, wxt, "x")
                idxf = tmp.tile([128, K2], F32, name="idxf", tag="idxf")
                nc.vector.scalar_tensor_tensor(idxf[:], gy[:], 67.0, gx[:],
                                               OP.mult, OP.add)
                nc.vector.tensor_copy(_sub(icol, 36 * qc + sl, [[4, K2]]),
                                      idxf[:])

        def main_quarter(qc):
            qps = []
            acc = [None, None]
            for k in range(K2):
                g = gpool.tile([128, 4096], F16, name="g", tag="g")
                if qc == 0 and k == 1:
                    # delayed weight load: WAW dep on the dummy write keeps
                    # this transfer out of the critical early DMA window
                    nc.vector.tensor_copy(dw[:, 0:2], icol[:, 0:2])
                    nc.sync.dma_start(dw[:], dwt.ap())
                for sl in range(4):
                    nc.gpsimd.indirect_dma_start(
                        out=g[:, sl * 1024:(sl + 1) * 1024], out_offset=None,
                        in_=xq_ap,
                        in_offset=bass.IndirectOffsetOnAxis(
                            ap=icol[:, 36 * qc + 4 * k + sl:
                                    36 * qc + 4 * k + sl + 1],
                            axis=0))
                slab = [slabs.tile([128, 512], F32, name=f"sl{chi}",
                                   tag=f"sl{chi}") for chi in range(2)]
                for sl in range(4):
                    col = k * 16 + 4 * qc + sl
                    s0 = sl * 1024
                    u = wpool.tile([128, 256], F16, name="u", tag="u")
                    nc.vector.scalar_tensor_tensor(
                        u[:], g[:, s0 + 768:s0 + 1024], wxt[:, col:col + 1],
                        g[:, s0 + 512:s0 + 768], OP.mult, OP.add)
                    dwx = wpool.tile([128, 128], F16, name="dwx", tag="dwx")
                    nc.vector.tensor_scalar(dwx[:], idt[:],
                                            wxt[:, col:col + 1], None,
                                            OP.mult)
                    dwy = wpool.tile([128, 128], F16, name="dwy", tag="dwy")
                    nc.vector.tensor_scalar(dwy[:], idt[:],
                                            wyt[:, col:col + 1], None,
                                            OP.mult)
                    for chi in range(2):
                        dst = slab[chi][:, sl * 128:(sl + 1) * 128]
                        nc.tensor.matmul(dst, g[:, s0 + chi * 128:
                                                s0 + chi * 128 + 128],
                                         idt[:], start=True, stop=False)
                        nc.tensor.matmul(dst, g[:, s0 + 256 + chi * 128:
                                                s0 + 256 + chi * 128 + 128],
                                         dwx[:], start=False, stop=False)
                        nc.tensor.matmul(dst, u[:, chi * 128:chi * 128 + 128],
                                         dwy[:], start=False, stop=True)
                for chi in range(2):
                    qp = qpool.tile([128, 512], F16, name="qp", tag="qp")
                    if k == 8 and chi == 1:
                        # last tap: run the two slab copies on ACT and DVE in
                        # parallel to shorten the tail chain
                        nc.vector.tensor_copy(qp[:], slab[chi][:])
                    else:
                        nc.scalar.activation(qp[:], slab[chi][:], AF.Copy)
                    qps.append((k, chi, qp))
                # main matmul in closed partial groups (after taps 2, 5, 8)
                # accumulated into SBUF so the last tap leaves only a 12-mm
                # tail instead of 36
                if k in (2, 5, 7, 8):
                    for oc in range(2):
                        po = outps.tile([128, 512], F32, name=f"po{oc}",
                                        tag="po")
                        n = 0
                        for kk, chi, qp in qps:
                            lhsT = _sub(dw, ((kk * 2 + chi) * 2 + oc) * 128,
                                        [[1, 128]])
                            nc.tensor.matmul(po[:], lhsT, qp[:],
                                             start=(n == 0),
                                             stop=(n == len(qps) - 1))
                            n += 1
                        if k == 2:
                            acc[oc] = bpool.tile([128, 512], F32,
                                                 name=f"acc{oc}",
                                                 tag=f"acc{oc}")
                            nc.vector.tensor_scalar(acc[oc][:], po[:],
                                                    db[:, oc:oc + 1], None,
                                                    OP.add)
                        elif k < 8:
                            nc.vector.tensor_add(acc[oc][:], acc[oc][:],
                                                 po[:])
                        else:
                            a16 = bpool.tile([128, 512], F16,
                                             name=f"a16{oc}", tag=f"a16{oc}")
                            nc.vector.tensor_add(a16[:], acc[oc][:], po[:])
                            nc.sync.dma_start(
                                out.ap()[oc * 128:(oc + 1) * 128,
                                         qc * 512:(qc + 1) * 512], a16[:])
                    qps = []

        for qc in range(4):
            head_quarter(qc)
        for qc in range(4):
            main_quarter(qc)


def _prep_host(inputs):
    """Per-core input maps (host does layout only)."""
    x = np.asarray(inputs["x"], np.float32)
    y = np.asarray(inputs["y"], np.float32)
    offw = np.asarray(inputs["offset_w"], np.float32)
    offb = np.asarray(inputs["offset_b"], np.float32)
    dww = np.asarray(inputs["deform_w"], np.float32)
    dbb = np.asarray(inputs["deform_b"], np.float32)

    ow = np.zeros((128, 18, 18), np.float16)
    wr = offw.reshape(18, 2, 128, 3, 3)
    for k in range(9):
        for chi in range(2):
            ow[:, k * 2 + chi, :] = wr[:, chi, :, k // 3, k % 3].T
    dwm = np.zeros((128, 36, 128), np.float16)
    dr = dww.reshape(2, 128, 2, 128, 3, 3)
    for k in range(9):
        for chi in range(2):
            for oc in range(2):
                dwm[:, (k * 2 + chi) * 2 + oc, :] = \
                    dr[oc, :, chi, :, k // 3, k % 3].T

    dbias = dbb.reshape(2, 128).T.astype(np.float32).copy()
    offbt = offb.reshape(18, 1).astype(np.float32)
    ident = np.eye(128, dtype=np.float16)

    # padded a/b/c/d corner table per batch: grid (gy,gx) in [-2,64]
    quads = []
    for b in range(B):
        xp = np.zeros((H + 4, W + 4, C), np.float32)
        xp[2:2 + H, 2:2 + W] = x[b].transpose(1, 2, 0)
        z00 = xp[0:GRID, 0:GRID]
        z01 = xp[0:GRID, 1:GRID + 1]
        z10 = xp[1:GRID + 1, 0:GRID]
        z11 = xp[1:GRID + 1, 1:GRID + 1]
        q = np.empty((GRID, GRID, 4, C), np.float16)
        q[:, :, 0] = z00
        q[:, :, 1] = z01 - z00
        q[:, :, 2] = z10 - z00
        q[:, :, 3] = z11 - z01 - z10 + z00
        quads.append(q.reshape(NROW, 1024))

    # base grids in [p, (k, s)] layout, +2 pad-shift baked in
    pv = np.arange(128)
    sv = np.arange(16)
    kiv = (np.arange(9) // 3).astype(np.float32)
    kjv = (np.arange(9) % 3).astype(np.float32)
    pxg = sv[None, :] * 128 + pv[:, None]           # [p, s]
    wg = (pxg % 64).astype(np.float32)
    hg = (pxg // 64).astype(np.float32)
    bxg = np.zeros((128, 9, 16), np.float32)
    for k in range(9):
        bxg[:, k, :] = wg + (kjv[k] - 1.0) + 2.0

    in_maps = []
    for core in range(8):
        b, r = core // 2, core % 2
        yp = np.zeros((128, 2, 34, 66), np.float16)
        lo = r * HALF
        slo, shi = max(lo - 1, 0), min(lo + HALF + 1, H)
        ys = y[b, :, slo:shi, :].reshape(2, 128, shi - slo, W)
        yp[:, :, (slo - lo + 1):(shi - lo + 1), 1:65] = ys.transpose(1, 0, 2, 3)

        byw = np.zeros((128, 9, 16), np.float32)
        for k in range(9):
            byw[:, k, :] = r * HALF + hg + (kiv[k] - 1.0) + 2.0

        in_maps.append({
            "xq": quads[b],
            "ypad": yp.reshape(128, 4488),
            "offw": ow.reshape(128, 324),
            "dwt": dwm.reshape(128, 4608),
            "offb": offbt,
            "dbias": dbias,
            "basey": byw.reshape(128, 144),
            "basex": bxg.reshape(128, 144),
            "ident": ident,
        })
    return in_maps


def kernel(**inputs) -> np.ndarray:
    if "nc" not in _cache:
        _cache["nc"] = build_nc()
    nc = _cache["nc"]
    in_maps = _prep_host(inputs)
    res = run_bass_kernel_spmd(nc, in_maps, core_ids=list(range(8)))

    out = np.zeros((B, O, H, W), np.float32)
    for core in range(8):
        b, r = core // 2, core % 2
        o = res.results[core]["out"]          # [256, 2048] f16, cols = px
        out[b, :, r * HALF:(r + 1) * HALF, :] = \
            o.astype(np.float32).reshape(O, HALF, W)
    return out


if __name__ == "__main__":
    nc = build_nc()
    print("build OK")
